# revision 5
# baseline (speedup 1.0000x reference)
"""Trainium2 Bass kernel for NeuralGraphHidden (GNN message passing).

Math (per molecule b, atom a):
    deg[b,a]    = #valid edges (edges[b,a,:] != -1)
    summed_atom = atoms[b,a] + sum_s atoms[b, edges[b,a,s]]          (64)
    bond_sum    = sum_s bonds[b,a,s]                                  (8)
    x           = concat(summed_atom, bond_sum)                      (72)
    out[b,a]    = relu(x @ Ws[deg] + bs[deg])  if deg <= 5 else 0   (128)

Design notes (driven by measured TRN2 behaviour on this system):
  * Device-side random-row gathers measured 20-500 ns/row -> the host does
    all *layout* work (degree-sort permutation, neighbour row expansion via
    np.take, bf16 packing, feature-major transposes), which is pure indexed
    data movement; the device does all arithmetic.
  * Everything is delivered FEATURE-MAJOR (partition = feature, free = sorted
    token slot), so the device needs no transposes at all:
      - xrowsT  [112, 15360]: rows 0:64 self atom features, rows 64:112 the
        six raw bond vectors; the bond sum happens inside the matmul because
        Wb is tiled 6x along K in wfull.
      - ncatT   [64, 38400]: neighbour atom features in process order
        (degree ascending, slot s within degree), each region [64, 2560].
      - out[c, tok] = relu(Wd^T x + b) with conv on PARTITIONS, so the bias
        is a per-partition scalar folded into the Scalar-engine relu.
  * Per degree group d (2560 slots): neighbour slots s<3 are summed into the
    self rows by three wide DVE adds; slots s>=3 are folded into the main
    matmul via PSUM accumulation (lhsT = atom part of the weights). One
    matmul streams a 512-col quad (one PSUM bank); ~80 instructions total.
  * DMA is the roofline (~12.5 MB/core): loads are coalesced into a few
    multi-group transfers (10-46 KB per descriptor - small descriptors
    measured ~35% slower per byte), issued from two queues so descriptor
    generation (~1.2 us per dma_start) doesn't serialize the ramp; stores
    go out every two groups.
  * Host unpermutes the sorted output (deg-6 rows are zero).
"""

import sys

sys.path.insert(0, "/opt/trn_rl_repo")

import numpy as np
import ml_dtypes

from contextlib import ExitStack

import concourse.bacc as bacc
import concourse.tile as tile
from concourse import mybir
from concourse.bass_utils import run_bass_kernel_spmd

# Problem shapes (hardcoded per the harness contract).
B, A, D = 1024, 128, 6
F_ATOM, F_BOND, CONV = 64, 8, 128
NCORES = 8
BS = B // NCORES          # molecules per core = 128
T = BS * A                # tokens per core = 16384
ROW = F_ATOM + D * F_BOND               # 112 features per packed row
GROUP_PAD = 2560                        # per-degree group size (static)
NSORT = D * GROUP_PAD                   # 15360 sorted slots
QW = 512                                # quad width (one PSUM bank of f32)
NQ = GROUP_PAD // QW                    # 5 quads per group
S_DVE = 3                               # neighbour slots s < S_DVE go to DVE
# group d (degree d) occupies slot columns [d*GROUP_PAD, (d+1)*GROUP_PAD);
# ncat region (d, s) starts at column RCOL[d] + s*GROUP_PAD
RCOL = [0]
for d in range(D):
    RCOL.append(RCOL[-1] + d * GROUP_PAD)
NCAT_COLS = RCOL[D]                     # 38400
# chunked loads: xrowsT/osortT in 3 chunks of 2 groups, ncatT in 3 chunks
XCHUNKS = [(0, 2), (2, 4), (4, 6)]      # [d0, d1) group ranges
NCHUNKS = [(1, 3), (3, 5), (5, 6)]

_f32 = mybir.dt.float32
_bf16 = mybir.dt.bfloat16

_cached = {}


def build_program():
    """Build the (static) per-core Bass/Tile program."""
    nc = bacc.Bacc("TRN2", target_bir_lowering=False, debug=False)

    xrowsT = nc.dram_tensor("xrowsT", [ROW, NSORT], _bf16, kind="ExternalInput")
    ncatT = nc.dram_tensor("ncatT", [F_ATOM, NCAT_COLS], _bf16,
                           kind="ExternalInput")
    wpack = nc.dram_tensor("wpack", [ROW, D * CONV], _bf16,
                           kind="ExternalInput")
    bsT = nc.dram_tensor("bsT", [CONV, D], _f32, kind="ExternalInput")
    osortT = nc.dram_tensor("osortT", [CONV, NSORT], _bf16,
                            kind="ExternalOutput")

    with tile.TileContext(nc) as tc, ExitStack() as ctx:
        const_pool = ctx.enter_context(tc.tile_pool(name="const", bufs=1))
        work_pool = ctx.enter_context(tc.tile_pool(name="work", bufs=1))
        ps_pool = ctx.enter_context(tc.tile_pool(name="ps", bufs=8,
                                                 space="PSUM"))

        # Loads, split across two issuing queues so descriptor generation
        # overlaps; earliest-needed data first on each queue.
        wp_t = const_pool.tile([ROW, D * CONV], _bf16, tag="wpack")
        bs_t = const_pool.tile([CONV, D], _f32, tag="bsT")
        xt_t, nc_t = {}, {}
        for (d0, d1) in XCHUNKS:
            xt_t[d0] = work_pool.tile([ROW, (d1 - d0) * GROUP_PAD], _bf16,
                                      tag=f"xt{d0}", name=f"xt{d0}")
        for (d0, d1) in NCHUNKS:
            nc_t[d0] = work_pool.tile(
                [F_ATOM, RCOL[d1] - RCOL[d0]], _bf16, tag=f"nc{d0}",
                name=f"nc{d0}")

        nc.sync.dma_start(out=wp_t[:], in_=wpack[:])
        nc.scalar.dma_start(out=bs_t[:], in_=bsT[:])
        for i, (d0, d1) in enumerate(XCHUNKS):
            eng = nc.sync if i % 2 == 0 else nc.scalar
            eng.dma_start(
                out=xt_t[d0][:],
                in_=xrowsT[:, d0 * GROUP_PAD:d1 * GROUP_PAD])
        for i, (d0, d1) in enumerate(NCHUNKS):
            eng = nc.scalar if i % 2 == 0 else nc.sync
            eng.dma_start(out=nc_t[d0][:], in_=ncatT[:, RCOL[d0]:RCOL[d1]])

        def xt_ap(d):
            """Group-d self rows [112, GROUP_PAD] inside its load chunk."""
            d0 = next(c0 for (c0, c1) in XCHUNKS if c0 <= d < c1)
            off = (d - d0) * GROUP_PAD
            return xt_t[d0][:, off:off + GROUP_PAD]

        def ncat_ap(d, s, cols):
            """Neighbour region (d, s) columns `cols` inside its chunk."""
            d0 = next(c0 for (c0, c1) in NCHUNKS if c0 <= d < c1)
            off = RCOL[d] - RCOL[d0] + s * GROUP_PAD
            return nc_t[d0][:, off + cols.start:off + cols.stop]

        out_t = {}
        for d in range(D):
            xt = xt_ap(d)
            for s in range(min(d, S_DVE)):
                nc.vector.tensor_add(
                    xt[0:F_ATOM, :], xt[0:F_ATOM, :],
                    ncat_ap(d, s, slice(0, GROUP_PAD)))
            if d % 2 == 0:
                out_t[d] = work_pool.tile([CONV, 2 * GROUP_PAD], _bf16,
                                          tag=f"out{d}", name=f"out{d}")
            out_g = out_t[d - d % 2]
            ocol = (d % 2) * GROUP_PAD
            n_pe = max(0, d - S_DVE)
            wmain = wp_t[:, d * CONV:(d + 1) * CONV]
            watom = wp_t[0:F_ATOM, d * CONV:(d + 1) * CONV]
            for q in range(NQ):
                cols = slice(q * QW, (q + 1) * QW)
                ps = ps_pool.tile([CONV, QW], _f32, tag="ps")
                nc.tensor.matmul(out=ps[:], lhsT=wmain, rhs=xt[:, cols],
                                 start=True, stop=(n_pe == 0))
                for j, s in enumerate(range(S_DVE, d)):
                    nc.tensor.matmul(out=ps[:], lhsT=watom,
                                     rhs=ncat_ap(d, s, cols),
                                     start=False, stop=(j == n_pe - 1))
                nc.scalar.activation(out_g[:, ocol + cols.start:
                                           ocol + cols.stop], ps[:],
                                     mybir.ActivationFunctionType.Relu,
                                     bias=bs_t[:, d:d + 1])
            if d % 2 == 1:
                eng = nc.sync if d == 1 else nc.scalar
                eng.dma_start(
                    out=osortT[:, (d - 1) * GROUP_PAD:(d + 1) * GROUP_PAD],
                    in_=out_g[:])

    nc.compile()
    return nc


def _get_program():
    if "nc" not in _cached:
        _cached["nc"] = build_program()
    return _cached["nc"]


def prep_core_inputs(atoms_s, bonds_s, edges_s, wpack_np, bsT_np):
    """Host-side layout/index prep for one core's shard (numpy only)."""
    deg = (edges_s != -1).sum(axis=-1).reshape(-1)            # [T] natural
    slot_tok = np.full(NSORT, -1, np.int64)   # sorted slot -> natural token
    for d in range(D):
        toks = np.nonzero(deg == d)[0]
        n = len(toks)
        assert n <= GROUP_PAD, f"degree-{d} group has {n} > {GROUP_PAD}"
        slot_tok[d * GROUP_PAD:d * GROUP_PAD + n] = toks

    flat = np.concatenate(
        [atoms_s.reshape(T, F_ATOM), bonds_s.reshape(T, D * F_BOND)], axis=1
    ).astype(ml_dtypes.bfloat16)                              # [T, 112]
    safe = np.maximum(slot_tok, 0)
    xrows = np.where((slot_tok >= 0)[:, None], flat[safe],
                     ml_dtypes.bfloat16(0))                   # [NSORT, 112]
    xrowsT = np.ascontiguousarray(xrows.T)                    # [112, NSORT]

    eflat = edges_s.reshape(T, D)
    bcol = (np.arange(T) // A) * A                            # molecule base
    atoms_flat = flat[:, :F_ATOM]
    regions = []
    for d in range(D):
        slots = slot_tok[d * GROUP_PAD:(d + 1) * GROUP_PAD]
        sv = slots >= 0
        st = np.maximum(slots, 0)
        for s in range(d):
            e = np.where(sv, eflat[st, s], -1)
            nat = np.maximum(bcol[st] + e, 0)
            regions.append(np.where((e >= 0)[:, None], atoms_flat[nat],
                                    ml_dtypes.bfloat16(0)))   # [2560, 64]
    ncat = np.concatenate(regions, axis=0)                    # [38400, 64]
    ncatT = np.ascontiguousarray(ncat.T)                      # [64, 38400]

    return {
        "xrowsT": xrowsT,
        "ncatT": ncatT,
        "wpack": wpack_np,
        "bsT": bsT_np,
    }, slot_tok


def kernel(atoms, bonds, edges, Ws, bs, trace=False):
    atoms = np.asarray(atoms)
    bonds = np.asarray(bonds)
    edges = np.asarray(edges)
    Ws = np.asarray(Ws)
    bs = np.asarray(bs)

    # wpack[:, d*128:(d+1)*128] = [Wa_d (64) | tile(Wb_d, 6) (48)]; the 6x
    # tiling makes the matmul itself perform the bond sum.
    wfull = np.zeros((D, ROW, CONV), np.float32)
    wfull[:, :F_ATOM] = Ws[:, :F_ATOM]
    wfull[:, F_ATOM:] = np.tile(Ws[:, F_ATOM:], (1, D, 1))
    wpack_np = np.ascontiguousarray(
        wfull.transpose(1, 0, 2).reshape(ROW, D * CONV)
    ).astype(ml_dtypes.bfloat16)
    bsT_np = np.ascontiguousarray(bs.T.astype(np.float32))    # [128, 6]

    in_maps, slot_toks = [], []
    for c in range(NCORES):
        sl = slice(c * BS, (c + 1) * BS)
        m, st = prep_core_inputs(atoms[sl], bonds[sl], edges[sl],
                                 wpack_np, bsT_np)
        in_maps.append(m)
        slot_toks.append(st)

    nc = _get_program()
    res = run_bass_kernel_spmd(nc, in_maps, core_ids=list(range(NCORES)),
                               trace=trace)
    kernel.last_results = res

    out = np.zeros((B, A, CONV), np.float32)
    for c in range(NCORES):
        osortT = res.results[c]["osortT"].view(ml_dtypes.bfloat16)
        osort = osortT.reshape(CONV, NSORT).T                 # [NSORT, 128]
        st = slot_toks[c]
        real = st >= 0
        shard = out[c * BS:(c + 1) * BS].reshape(T, CONV)
        shard[st[real]] = osort[real].astype(np.float32)
    return out


# revision 11
# speedup vs baseline: 1.1812x; 1.1812x over previous
"""Trainium2 Bass kernel for NeuralGraphHidden (GNN message passing).

Math (per molecule b, atom a):
    deg[b,a]    = #valid edges (edges[b,a,:] != -1)
    summed_atom = atoms[b,a] + sum_s atoms[b, edges[b,a,s]]          (64)
    bond_sum    = sum_s bonds[b,a,s]                                  (8)
    x           = concat(summed_atom, bond_sum)                      (72)
    out[b,a]    = relu(x @ Ws[deg] + bs[deg])  if deg <= 5 else 0   (128)

Design notes (driven by measured TRN2 behaviour on this system):
  * Device-side random-row gathers measured 20-500 ns/row -> the host does
    all *layout* work (degree-sort permutation, neighbour row expansion via
    np.take, bf16 packing, feature-major transposes), which is pure indexed
    data movement; the device does all arithmetic.
  * Everything is delivered FEATURE-MAJOR (partition = feature, free = sorted
    token slot), so the device needs no transposes at all:
      - xrowsT  [112, 15360]: rows 0:64 self atom features, rows 64:112 the
        six raw bond vectors; the bond sum happens inside the matmul because
        Wb is tiled 6x along K in wfull.
      - ncatT   [64, 38400]: neighbour atom features in process order
        (degree ascending, slot s within degree), each region [64, 2560].
      - out[c, tok] = relu(Wd^T x + b) with conv on PARTITIONS, so the bias
        is a per-partition scalar folded into the Scalar-engine relu.
  * Per degree group d (2560 slots): neighbour slots s<3 are summed into the
    self rows by three wide DVE adds; slots s>=3 are folded into the main
    matmul via PSUM accumulation (lhsT = atom part of the weights). One
    matmul streams a 512-col quad (one PSUM bank); ~80 instructions total.
  * DMA is the roofline (~12.5 MB/core): loads are coalesced into a few
    multi-group transfers (10-46 KB per descriptor - small descriptors
    measured ~35% slower per byte), issued from two queues so descriptor
    generation (~1.2 us per dma_start) doesn't serialize the ramp; stores
    go out every two groups.
  * Host unpermutes the sorted output (deg-6 rows are zero).
"""

import sys

sys.path.insert(0, "/opt/trn_rl_repo")

import numpy as np
import ml_dtypes

from contextlib import ExitStack

import concourse.bacc as bacc
import concourse.tile as tile
from concourse import mybir
from concourse.bass_utils import run_bass_kernel_spmd

# Problem shapes (hardcoded per the harness contract).
B, A, D = 1024, 128, 6
F_ATOM, F_BOND, CONV = 64, 8, 128
NCORES = 8
BS = B // NCORES          # molecules per core = 128
T = BS * A                # tokens per core = 16384
ROW = F_ATOM + D * F_BOND               # 112 features per packed row
GROUP_PAD = 2560                        # per-degree group size (static)
NSORT = D * GROUP_PAD                   # 15360 sorted slots
QW = 512                                # quad width (one PSUM bank of f32)
NQ = GROUP_PAD // QW                    # 5 quads per group
S_DVE = 3                               # neighbour slots s < S_DVE go to DVE
# group d (degree d) occupies slot columns [d*GROUP_PAD, (d+1)*GROUP_PAD);
# ncat region (d, s) starts at column RCOL[d] + s*GROUP_PAD
RCOL = [0]
for d in range(D):
    RCOL.append(RCOL[-1] + d * GROUP_PAD)
NCAT_COLS = RCOL[D]                     # 38400
# chunked loads: xrowsT/osortT in 3 chunks of 2 groups, ncatT in 3 chunks
XCHUNKS = [(4, 6), (2, 4), (0, 2)]      # [d0, d1) group ranges, load order
NCHUNKS = [(5, 6), (3, 5), (1, 3)]
DORDER = [5, 4, 3, 2, 1, 0]             # process heaviest degree first

_f32 = mybir.dt.float32
_bf16 = mybir.dt.bfloat16

_cached = {}


def build_program():
    """Build the (static) per-core Bass/Tile program."""
    nc = bacc.Bacc("TRN2", target_bir_lowering=False, debug=False)

    xrowsT = nc.dram_tensor("xrowsT", [ROW, NSORT], _bf16, kind="ExternalInput")
    ncatT = nc.dram_tensor("ncatT", [F_ATOM, NCAT_COLS], _bf16,
                           kind="ExternalInput")
    wpack = nc.dram_tensor("wpack", [ROW, D * CONV], _bf16,
                           kind="ExternalInput")
    bsT = nc.dram_tensor("bsT", [CONV, D], _f32, kind="ExternalInput")
    osortT = nc.dram_tensor("osortT", [CONV, NSORT], _bf16,
                            kind="ExternalOutput")

    with tile.TileContext(nc) as tc, ExitStack() as ctx:
        const_pool = ctx.enter_context(tc.tile_pool(name="const", bufs=1))
        work_pool = ctx.enter_context(tc.tile_pool(name="work", bufs=1))
        ps_pool = ctx.enter_context(tc.tile_pool(name="ps", bufs=8,
                                                 space="PSUM"))

        # Loads, split across two issuing queues so descriptor generation
        # overlaps; earliest-needed data first on each queue.
        wp_t = const_pool.tile([ROW, D * CONV], _bf16, tag="wpack")
        bs_t = const_pool.tile([CONV, D], _f32, tag="bsT")
        xt_t, nc_t = {}, {}
        for (d0, d1) in XCHUNKS:
            xt_t[d0] = work_pool.tile([ROW, (d1 - d0) * GROUP_PAD], _bf16,
                                      tag=f"xt{d0}", name=f"xt{d0}")
        for (d0, d1) in NCHUNKS:
            nc_t[d0] = work_pool.tile(
                [F_ATOM, RCOL[d1] - RCOL[d0]], _bf16, tag=f"nc{d0}",
                name=f"nc{d0}")

        # Issue loads in CONSUMPTION order, round-robin across four queues:
        # the 16 DMA engines drain descriptors FIFO, so arrival order tracks
        # issue order and early groups' data lands first.
        loads = [("w", None), ("b", None),
                 ("x", XCHUNKS[0]), ("n", NCHUNKS[0]),   # d5
                 ("n", NCHUNKS[1]), ("x", XCHUNKS[1]),   # d4, d3
                 ("n", NCHUNKS[2]), ("x", XCHUNKS[2])]   # d2/d1, d0
        queues = [nc.sync, nc.scalar, nc.gpsimd]
        for i, (kind, rng) in enumerate(loads):
            eng = queues[i % 3]
            if kind == "w":
                eng.dma_start(out=wp_t[:], in_=wpack[:])
            elif kind == "b":
                eng.dma_start(out=bs_t[:], in_=bsT[:])
            elif kind == "x":
                d0, d1 = rng
                eng.dma_start(
                    out=xt_t[d0][:],
                    in_=xrowsT[:, d0 * GROUP_PAD:d1 * GROUP_PAD])
            else:
                d0, d1 = rng
                eng.dma_start(out=nc_t[d0][:], in_=ncatT[:, RCOL[d0]:RCOL[d1]])

        def xt_ap(d):
            """Group-d self rows [112, GROUP_PAD] inside its load chunk."""
            d0 = next(c0 for (c0, c1) in XCHUNKS if c0 <= d < c1)
            off = (d - d0) * GROUP_PAD
            return xt_t[d0][:, off:off + GROUP_PAD]

        def ncat_ap(d, s, cols):
            """Neighbour region (d, s) columns `cols` inside its chunk."""
            d0 = next(c0 for (c0, c1) in NCHUNKS if c0 <= d < c1)
            off = RCOL[d] - RCOL[d0] + s * GROUP_PAD
            return nc_t[d0][:, off + cols.start:off + cols.stop]

        # stores: 2-group for the early (heavy) groups, single-group for the
        # final two so the drain tail after the last relu is short
        STORES = {4: (4, 6), 2: (2, 4), 1: (1, 2), 0: (0, 1)}
        out_t = {}
        for od, (d0, d1) in STORES.items():
            out_t[od] = work_pool.tile([CONV, (d1 - d0) * GROUP_PAD], _bf16,
                                       tag=f"out{od}", name=f"out{od}")

        def out_ap(d):
            od = next(o for o, (c0, c1) in STORES.items() if c0 <= d < c1)
            off = (d - STORES[od][0]) * GROUP_PAD
            return out_t[od], od, off

        for i, d in enumerate(DORDER):
            xt = xt_ap(d)
            for s in range(min(d, S_DVE)):
                nc.vector.tensor_add(
                    xt[0:F_ATOM, :], xt[0:F_ATOM, :],
                    ncat_ap(d, s, slice(0, GROUP_PAD)))
            out_g, od, ocol = out_ap(d)
            n_pe = max(0, d - S_DVE)
            wmain = wp_t[:, d * CONV:(d + 1) * CONV]
            watom = wp_t[0:F_ATOM, d * CONV:(d + 1) * CONV]
            for q in range(NQ):
                cols = slice(q * QW, (q + 1) * QW)
                ps = ps_pool.tile([CONV, QW], _f32, tag="ps")
                nc.tensor.matmul(out=ps[:], lhsT=wmain, rhs=xt[:, cols],
                                 start=True, stop=(n_pe == 0))
                for j, s in enumerate(range(S_DVE, d)):
                    nc.tensor.matmul(out=ps[:], lhsT=watom,
                                     rhs=ncat_ap(d, s, cols),
                                     start=False, stop=(j == n_pe - 1))
                nc.scalar.activation(out_g[:, ocol + cols.start:
                                           ocol + cols.stop], ps[:],
                                     mybir.ActivationFunctionType.Relu,
                                     bias=bs_t[:, d:d + 1])
            if d == od:
                c0, c1 = STORES[od]
                eng = queues[i % 3]
                eng.dma_start(
                    out=osortT[:, c0 * GROUP_PAD:c1 * GROUP_PAD],
                    in_=out_t[od][:])

    nc.compile()
    return nc


def _get_program():
    if "nc" not in _cached:
        _cached["nc"] = build_program()
    return _cached["nc"]


def prep_core_inputs(atoms_s, bonds_s, edges_s, wpack_np, bsT_np):
    """Host-side layout/index prep for one core's shard (numpy only)."""
    deg = (edges_s != -1).sum(axis=-1).reshape(-1)            # [T] natural
    slot_tok = np.full(NSORT, -1, np.int64)   # sorted slot -> natural token
    for d in range(D):
        toks = np.nonzero(deg == d)[0]
        n = len(toks)
        assert n <= GROUP_PAD, f"degree-{d} group has {n} > {GROUP_PAD}"
        slot_tok[d * GROUP_PAD:d * GROUP_PAD + n] = toks

    flat = np.concatenate(
        [atoms_s.reshape(T, F_ATOM), bonds_s.reshape(T, D * F_BOND)], axis=1
    ).astype(ml_dtypes.bfloat16)                              # [T, 112]
    safe = np.maximum(slot_tok, 0)
    xrows = np.where((slot_tok >= 0)[:, None], flat[safe],
                     ml_dtypes.bfloat16(0))                   # [NSORT, 112]
    xrowsT = np.ascontiguousarray(xrows.T)                    # [112, NSORT]

    eflat = edges_s.reshape(T, D)
    bcol = (np.arange(T) // A) * A                            # molecule base
    atoms_flat = flat[:, :F_ATOM]
    regions = []
    for d in range(D):
        slots = slot_tok[d * GROUP_PAD:(d + 1) * GROUP_PAD]
        sv = slots >= 0
        st = np.maximum(slots, 0)
        for s in range(d):
            e = np.where(sv, eflat[st, s], -1)
            nat = np.maximum(bcol[st] + e, 0)
            regions.append(np.where((e >= 0)[:, None], atoms_flat[nat],
                                    ml_dtypes.bfloat16(0)))   # [2560, 64]
    ncat = np.concatenate(regions, axis=0)                    # [38400, 64]
    ncatT = np.ascontiguousarray(ncat.T)                      # [64, 38400]

    return {
        "xrowsT": xrowsT,
        "ncatT": ncatT,
        "wpack": wpack_np,
        "bsT": bsT_np,
    }, slot_tok


def kernel(atoms, bonds, edges, Ws, bs, trace=False):
    atoms = np.asarray(atoms)
    bonds = np.asarray(bonds)
    edges = np.asarray(edges)
    Ws = np.asarray(Ws)
    bs = np.asarray(bs)

    # wpack[:, d*128:(d+1)*128] = [Wa_d (64) | tile(Wb_d, 6) (48)]; the 6x
    # tiling makes the matmul itself perform the bond sum.
    wfull = np.zeros((D, ROW, CONV), np.float32)
    wfull[:, :F_ATOM] = Ws[:, :F_ATOM]
    wfull[:, F_ATOM:] = np.tile(Ws[:, F_ATOM:], (1, D, 1))
    wpack_np = np.ascontiguousarray(
        wfull.transpose(1, 0, 2).reshape(ROW, D * CONV)
    ).astype(ml_dtypes.bfloat16)
    bsT_np = np.ascontiguousarray(bs.T.astype(np.float32))    # [128, 6]

    in_maps, slot_toks = [], []
    for c in range(NCORES):
        sl = slice(c * BS, (c + 1) * BS)
        m, st = prep_core_inputs(atoms[sl], bonds[sl], edges[sl],
                                 wpack_np, bsT_np)
        in_maps.append(m)
        slot_toks.append(st)

    nc = _get_program()
    res = run_bass_kernel_spmd(nc, in_maps, core_ids=list(range(NCORES)),
                               trace=trace)
    kernel.last_results = res

    out = np.zeros((B, A, CONV), np.float32)
    for c in range(NCORES):
        osortT = res.results[c]["osortT"].view(ml_dtypes.bfloat16)
        osort = osortT.reshape(CONV, NSORT).T                 # [NSORT, 128]
        st = slot_toks[c]
        real = st >= 0
        shard = out[c * BS:(c + 1) * BS].reshape(T, CONV)
        shard[st[real]] = osort[real].astype(np.float32)
    return out


# revision 13
# speedup vs baseline: 1.5088x; 1.2774x over previous
"""Trainium2 Bass kernel for NeuralGraphHidden (GNN message passing).

Math (per molecule b, atom a):
    deg[b,a]    = #valid edges (edges[b,a,:] != -1)
    summed_atom = atoms[b,a] + sum_s atoms[b, edges[b,a,s]]          (64)
    bond_sum    = sum_s bonds[b,a,s]                                  (8)
    x           = concat(summed_atom, bond_sum)                      (72)
    out[b,a]    = relu(x @ Ws[deg] + bs[deg])  if deg <= 5 else 0   (128)

Design notes (driven by measured TRN2 behaviour on this system):
  * Device-side random-row gathers measured 20-500 ns/row -> the host does
    all *layout* work (degree-sort permutation, neighbour row expansion via
    np.take, bf16 packing, feature-major transposes), which is pure indexed
    data movement; the device does all arithmetic.
  * Everything is delivered FEATURE-MAJOR (partition = feature, free = sorted
    token slot), so the device needs no transposes at all:
      - xrowsT  [112, 15360]: rows 0:64 self atom features, rows 64:112 the
        six raw bond vectors; the bond sum happens inside the matmul because
        Wb is tiled 6x along K in wfull.
      - ncatT   [64, 38400]: neighbour atom features in process order
        (degree ascending, slot s within degree), each region [64, 2560].
      - out[c, tok] = relu(Wd^T x + b) with conv on PARTITIONS, so the bias
        is a per-partition scalar folded into the Scalar-engine relu.
  * Per degree group d (2560 slots): neighbour slots s<3 are summed into the
    self rows by three wide DVE adds; slots s>=3 are folded into the main
    matmul via PSUM accumulation (lhsT = atom part of the weights). One
    matmul streams a 512-col quad (one PSUM bank); ~80 instructions total.
  * DMA is the roofline (~12.5 MB/core): loads are coalesced into a few
    multi-group transfers (10-46 KB per descriptor - small descriptors
    measured ~35% slower per byte), issued from two queues so descriptor
    generation (~1.2 us per dma_start) doesn't serialize the ramp; stores
    go out every two groups.
  * Host unpermutes the sorted output (deg-6 rows are zero).
"""

import sys

sys.path.insert(0, "/opt/trn_rl_repo")

import numpy as np
import ml_dtypes

from contextlib import ExitStack

import concourse.bacc as bacc
import concourse.tile as tile
from concourse import mybir
from concourse.bass_utils import run_bass_kernel_spmd

# Problem shapes (hardcoded per the harness contract).
B, A, D = 1024, 128, 6
F_ATOM, F_BOND, CONV = 64, 8, 128
NCORES = 8
BS = B // NCORES          # molecules per core = 128
T = BS * A                # tokens per core = 16384
ROW = F_ATOM + D * F_BOND               # 112 features per packed row
GROUP_PAD = 2560                        # per-degree group size (static)
NSORT = D * GROUP_PAD                   # 15360 sorted slots
QW = 512                                # quad width (one PSUM bank of f32)
NQ = GROUP_PAD // QW                    # 5 quads per group
S_DVE = 3                               # neighbour slots s < S_DVE go to DVE
# group d (degree d) occupies slot columns [d*GROUP_PAD, (d+1)*GROUP_PAD);
# ncat region (d, s) starts at column RCOL[d] + s*GROUP_PAD
RCOL = [0]
for d in range(D):
    RCOL.append(RCOL[-1] + d * GROUP_PAD)
NCAT_COLS = RCOL[D]                     # 38400
# chunked loads: xrowsT/osortT in 3 chunks of 2 groups, ncatT in 3 chunks
XCHUNKS = [(4, 6), (2, 4), (0, 2)]      # [d0, d1) group ranges, load order
NCHUNKS = [(5, 6), (3, 5), (1, 3)]
DORDER = [5, 4, 3, 2, 1, 0]             # process heaviest degree first

_f32 = mybir.dt.float32
_bf16 = mybir.dt.bfloat16

_cached = {}


def build_program():
    """Build the (static) per-core Bass/Tile program."""
    nc = bacc.Bacc("TRN2", target_bir_lowering=False, debug=False)

    xrowsT = nc.dram_tensor("xrowsT", [ROW, NSORT], _bf16, kind="ExternalInput")
    ncatT = nc.dram_tensor("ncatT", [F_ATOM, NCAT_COLS], _bf16,
                           kind="ExternalInput")
    wpack = nc.dram_tensor("wpack", [ROW, D * CONV], _bf16,
                           kind="ExternalInput")
    bsT = nc.dram_tensor("bsT", [CONV, D], _f32, kind="ExternalInput")
    osortT = nc.dram_tensor("osortT", [CONV, NSORT], _bf16,
                            kind="ExternalOutput")

    with tile.TileContext(nc) as tc, ExitStack() as ctx:
        const_pool = ctx.enter_context(tc.tile_pool(name="const", bufs=1))
        work_pool = ctx.enter_context(tc.tile_pool(name="work", bufs=1))
        ps_pool = ctx.enter_context(tc.tile_pool(name="ps", bufs=8,
                                                 space="PSUM"))

        # Loads, split across two issuing queues so descriptor generation
        # overlaps; earliest-needed data first on each queue.
        wp_t = const_pool.tile([ROW, D * CONV], _bf16, tag="wpack")
        bs_t = const_pool.tile([CONV, D], _f32, tag="bsT")
        xt_t, nc_t = {}, {}
        for (d0, d1) in XCHUNKS:
            xt_t[d0] = work_pool.tile([ROW, (d1 - d0) * GROUP_PAD], _bf16,
                                      tag=f"xt{d0}", name=f"xt{d0}")
        for (d0, d1) in NCHUNKS:
            nc_t[d0] = work_pool.tile(
                [F_ATOM, RCOL[d1] - RCOL[d0]], _bf16, tag=f"nc{d0}",
                name=f"nc{d0}")

        # Issue ALL DMAs on ONE ring (sync) in CONSUMPTION order: a single
        # ring drains descriptors FIFO at full aggregate bandwidth, so data
        # arrives exactly in issue order.  Splitting across issuing engines
        # measured ~40% slower (rings interleave on the 16 DMA engines) and
        # destroys the ordering.
        loads = [("w", None), ("b", None),
                 ("x", XCHUNKS[0]), ("n", NCHUNKS[0]),   # d5
                 ("n", NCHUNKS[1]), ("x", XCHUNKS[1]),   # d4, d3
                 ("n", NCHUNKS[2]), ("x", XCHUNKS[2])]   # d2/d1, d0
        for kind, rng in loads:
            if kind == "w":
                nc.sync.dma_start(out=wp_t[:], in_=wpack[:])
            elif kind == "b":
                nc.sync.dma_start(out=bs_t[:], in_=bsT[:])
            elif kind == "x":
                d0, d1 = rng
                nc.sync.dma_start(
                    out=xt_t[d0][:],
                    in_=xrowsT[:, d0 * GROUP_PAD:d1 * GROUP_PAD])
            else:
                d0, d1 = rng
                nc.sync.dma_start(out=nc_t[d0][:],
                                  in_=ncatT[:, RCOL[d0]:RCOL[d1]])

        def xt_ap(d):
            """Group-d self rows [112, GROUP_PAD] inside its load chunk."""
            d0 = next(c0 for (c0, c1) in XCHUNKS if c0 <= d < c1)
            off = (d - d0) * GROUP_PAD
            return xt_t[d0][:, off:off + GROUP_PAD]

        def ncat_ap(d, s, cols):
            """Neighbour region (d, s) columns `cols` inside its chunk."""
            d0 = next(c0 for (c0, c1) in NCHUNKS if c0 <= d < c1)
            off = RCOL[d] - RCOL[d0] + s * GROUP_PAD
            return nc_t[d0][:, off + cols.start:off + cols.stop]

        # stores: 2-group for the early (heavy) groups, single-group for the
        # final two so the drain tail after the last relu is short
        STORES = {4: (4, 6), 2: (2, 4), 1: (1, 2), 0: (0, 1)}
        out_t = {}
        for od, (d0, d1) in STORES.items():
            out_t[od] = work_pool.tile([CONV, (d1 - d0) * GROUP_PAD], _bf16,
                                       tag=f"out{od}", name=f"out{od}")

        def out_ap(d):
            od = next(o for o, (c0, c1) in STORES.items() if c0 <= d < c1)
            off = (d - STORES[od][0]) * GROUP_PAD
            return out_t[od], od, off

        for i, d in enumerate(DORDER):
            xt = xt_ap(d)
            for s in range(min(d, S_DVE)):
                nc.vector.tensor_add(
                    xt[0:F_ATOM, :], xt[0:F_ATOM, :],
                    ncat_ap(d, s, slice(0, GROUP_PAD)))
            out_g, od, ocol = out_ap(d)
            n_pe = max(0, d - S_DVE)
            wmain = wp_t[:, d * CONV:(d + 1) * CONV]
            watom = wp_t[0:F_ATOM, d * CONV:(d + 1) * CONV]
            for q in range(NQ):
                cols = slice(q * QW, (q + 1) * QW)
                ps = ps_pool.tile([CONV, QW], _f32, tag="ps")
                nc.tensor.matmul(out=ps[:], lhsT=wmain, rhs=xt[:, cols],
                                 start=True, stop=(n_pe == 0))
                for j, s in enumerate(range(S_DVE, d)):
                    nc.tensor.matmul(out=ps[:], lhsT=watom,
                                     rhs=ncat_ap(d, s, cols),
                                     start=False, stop=(j == n_pe - 1))
                nc.scalar.activation(out_g[:, ocol + cols.start:
                                           ocol + cols.stop], ps[:],
                                     mybir.ActivationFunctionType.Relu,
                                     bias=bs_t[:, d:d + 1])
            if d == od:
                c0, c1 = STORES[od]
                nc.sync.dma_start(
                    out=osortT[:, c0 * GROUP_PAD:c1 * GROUP_PAD],
                    in_=out_t[od][:])

    nc.compile()
    return nc


def _get_program():
    if "nc" not in _cached:
        _cached["nc"] = build_program()
    return _cached["nc"]


def prep_core_inputs(atoms_s, bonds_s, edges_s, wpack_np, bsT_np):
    """Host-side layout/index prep for one core's shard (numpy only)."""
    deg = (edges_s != -1).sum(axis=-1).reshape(-1)            # [T] natural
    slot_tok = np.full(NSORT, -1, np.int64)   # sorted slot -> natural token
    for d in range(D):
        toks = np.nonzero(deg == d)[0]
        n = len(toks)
        assert n <= GROUP_PAD, f"degree-{d} group has {n} > {GROUP_PAD}"
        slot_tok[d * GROUP_PAD:d * GROUP_PAD + n] = toks

    flat = np.concatenate(
        [atoms_s.reshape(T, F_ATOM), bonds_s.reshape(T, D * F_BOND)], axis=1
    ).astype(ml_dtypes.bfloat16)                              # [T, 112]
    safe = np.maximum(slot_tok, 0)
    xrows = np.where((slot_tok >= 0)[:, None], flat[safe],
                     ml_dtypes.bfloat16(0))                   # [NSORT, 112]
    xrowsT = np.ascontiguousarray(xrows.T)                    # [112, NSORT]

    eflat = edges_s.reshape(T, D)
    bcol = (np.arange(T) // A) * A                            # molecule base
    atoms_flat = flat[:, :F_ATOM]
    regions = []
    for d in range(D):
        slots = slot_tok[d * GROUP_PAD:(d + 1) * GROUP_PAD]
        sv = slots >= 0
        st = np.maximum(slots, 0)
        for s in range(d):
            e = np.where(sv, eflat[st, s], -1)
            nat = np.maximum(bcol[st] + e, 0)
            regions.append(np.where((e >= 0)[:, None], atoms_flat[nat],
                                    ml_dtypes.bfloat16(0)))   # [2560, 64]
    ncat = np.concatenate(regions, axis=0)                    # [38400, 64]
    ncatT = np.ascontiguousarray(ncat.T)                      # [64, 38400]

    return {
        "xrowsT": xrowsT,
        "ncatT": ncatT,
        "wpack": wpack_np,
        "bsT": bsT_np,
    }, slot_tok


def kernel(atoms, bonds, edges, Ws, bs, trace=False):
    atoms = np.asarray(atoms)
    bonds = np.asarray(bonds)
    edges = np.asarray(edges)
    Ws = np.asarray(Ws)
    bs = np.asarray(bs)

    # wpack[:, d*128:(d+1)*128] = [Wa_d (64) | tile(Wb_d, 6) (48)]; the 6x
    # tiling makes the matmul itself perform the bond sum.
    wfull = np.zeros((D, ROW, CONV), np.float32)
    wfull[:, :F_ATOM] = Ws[:, :F_ATOM]
    wfull[:, F_ATOM:] = np.tile(Ws[:, F_ATOM:], (1, D, 1))
    wpack_np = np.ascontiguousarray(
        wfull.transpose(1, 0, 2).reshape(ROW, D * CONV)
    ).astype(ml_dtypes.bfloat16)
    bsT_np = np.ascontiguousarray(bs.T.astype(np.float32))    # [128, 6]

    in_maps, slot_toks = [], []
    for c in range(NCORES):
        sl = slice(c * BS, (c + 1) * BS)
        m, st = prep_core_inputs(atoms[sl], bonds[sl], edges[sl],
                                 wpack_np, bsT_np)
        in_maps.append(m)
        slot_toks.append(st)

    nc = _get_program()
    res = run_bass_kernel_spmd(nc, in_maps, core_ids=list(range(NCORES)),
                               trace=trace)
    kernel.last_results = res

    out = np.zeros((B, A, CONV), np.float32)
    for c in range(NCORES):
        osortT = res.results[c]["osortT"].view(ml_dtypes.bfloat16)
        osort = osortT.reshape(CONV, NSORT).T                 # [NSORT, 128]
        st = slot_toks[c]
        real = st >= 0
        shard = out[c * BS:(c + 1) * BS].reshape(T, CONV)
        shard[st[real]] = osort[real].astype(np.float32)
    return out


# revision 15
# speedup vs baseline: 1.5878x; 1.0523x over previous
"""Trainium2 Bass kernel for NeuralGraphHidden (GNN message passing).

Math (per molecule b, atom a):
    deg[b,a]    = #valid edges (edges[b,a,:] != -1)
    summed_atom = atoms[b,a] + sum_s atoms[b, edges[b,a,s]]          (64)
    bond_sum    = sum_s bonds[b,a,s]                                  (8)
    x           = concat(summed_atom, bond_sum)                      (72)
    out[b,a]    = relu(x @ Ws[deg] + bs[deg])  if deg <= 5 else 0   (128)

Design notes (driven by measured TRN2 behaviour on this system):
  * Device-side random-row gathers measured 20-500 ns/row -> the host does
    all *layout* work (degree-sort permutation, neighbour row expansion via
    np.take, bf16 packing, feature-major transposes), which is pure indexed
    data movement; the device does all arithmetic.
  * Everything is delivered FEATURE-MAJOR (partition = feature, free = sorted
    token slot), so the device needs no transposes at all:
      - xrowsT  [112, 15360]: rows 0:64 self atom features, rows 64:112 the
        six raw bond vectors; the bond sum happens inside the matmul because
        Wb is tiled 6x along K in wfull.
      - ncatT   [64, 38400]: neighbour atom features in process order
        (degree ascending, slot s within degree), each region [64, 2560].
      - out[c, tok] = relu(Wd^T x + b) with conv on PARTITIONS, so the bias
        is a per-partition scalar folded into the Scalar-engine relu.
  * Per degree group d (2560 slots): neighbour slots s<3 are summed into the
    self rows by three wide DVE adds; slots s>=3 are folded into the main
    matmul via PSUM accumulation (lhsT = atom part of the weights). One
    matmul streams a 512-col quad (one PSUM bank); ~80 instructions total.
  * DMA is the roofline (~12.5 MB/core): loads are coalesced into a few
    multi-group transfers (10-46 KB per descriptor - small descriptors
    measured ~35% slower per byte), issued from two queues so descriptor
    generation (~1.2 us per dma_start) doesn't serialize the ramp; stores
    go out every two groups.
  * Host unpermutes the sorted output (deg-6 rows are zero).
"""

import sys

sys.path.insert(0, "/opt/trn_rl_repo")

import numpy as np
import ml_dtypes

from contextlib import ExitStack

import concourse.bacc as bacc
import concourse.tile as tile
from concourse import mybir
from concourse.bass_utils import run_bass_kernel_spmd

# Problem shapes (hardcoded per the harness contract).
B, A, D = 1024, 128, 6
F_ATOM, F_BOND, CONV = 64, 8, 128
NCORES = 8
BS = B // NCORES          # molecules per core = 128
T = BS * A                # tokens per core = 16384
ROW = F_ATOM + D * F_BOND               # 112 features per packed row
GROUP_PAD = 2560                        # per-degree group size (static)
NSORT = D * GROUP_PAD                   # 15360 sorted slots
QW = 512                                # quad width (one PSUM bank of f32)
NQ = GROUP_PAD // QW                    # 5 quads per group
S_DVE = 3                               # neighbour slots s < S_DVE go to DVE
# group d (degree d) occupies slot columns [d*GROUP_PAD, (d+1)*GROUP_PAD);
# ncat region (d, s) starts at column RCOL[d] + s*GROUP_PAD
RCOL = [0]
for d in range(D):
    RCOL.append(RCOL[-1] + d * GROUP_PAD)
NCAT_COLS = RCOL[D]                     # 38400
# chunked loads in consumption order; the last two x chunks are single
# groups so the tail (last-arriving data -> compute -> store) stays short
XCHUNKS = [(4, 6), (2, 4), (1, 2), (0, 1)]   # [d0, d1) group ranges
NCHUNKS = [(5, 6), (3, 5), (1, 3)]
DORDER = [5, 4, 3, 2, 1, 0]             # process heaviest degree first

_f32 = mybir.dt.float32
_bf16 = mybir.dt.bfloat16

_cached = {}


def build_program():
    """Build the (static) per-core Bass/Tile program."""
    nc = bacc.Bacc("TRN2", target_bir_lowering=False, debug=False)

    xrowsT = nc.dram_tensor("xrowsT", [ROW, NSORT], _bf16, kind="ExternalInput")
    ncatT = nc.dram_tensor("ncatT", [F_ATOM, NCAT_COLS], _bf16,
                           kind="ExternalInput")
    wpack = nc.dram_tensor("wpack", [ROW, D * CONV], _bf16,
                           kind="ExternalInput")
    bsT = nc.dram_tensor("bsT", [CONV, D], _f32, kind="ExternalInput")
    osortT = nc.dram_tensor("osortT", [CONV, NSORT], _bf16,
                            kind="ExternalOutput")

    with tile.TileContext(nc) as tc, ExitStack() as ctx:
        const_pool = ctx.enter_context(tc.tile_pool(name="const", bufs=1))
        work_pool = ctx.enter_context(tc.tile_pool(name="work", bufs=1))
        ps_pool = ctx.enter_context(tc.tile_pool(name="ps", bufs=8,
                                                 space="PSUM"))

        # Loads, split across two issuing queues so descriptor generation
        # overlaps; earliest-needed data first on each queue.
        wp_t = const_pool.tile([ROW, D * CONV], _bf16, tag="wpack")
        bs_t = const_pool.tile([CONV, D], _f32, tag="bsT")
        xt_t, nc_t = {}, {}
        for (d0, d1) in XCHUNKS:
            xt_t[d0] = work_pool.tile([ROW, (d1 - d0) * GROUP_PAD], _bf16,
                                      tag=f"xt{d0}", name=f"xt{d0}")
        for (d0, d1) in NCHUNKS:
            nc_t[d0] = work_pool.tile(
                [F_ATOM, RCOL[d1] - RCOL[d0]], _bf16, tag=f"nc{d0}",
                name=f"nc{d0}")

        # Issue ALL DMAs on ONE ring (sync) in CONSUMPTION order: a single
        # ring drains descriptors FIFO at full aggregate bandwidth, so data
        # arrives exactly in issue order.  Splitting across issuing engines
        # measured ~40% slower (rings interleave on the 16 DMA engines) and
        # destroys the ordering.
        loads = [("w", None),
                 ("x", XCHUNKS[0]), ("n", NCHUNKS[0]),   # d5
                 ("b", None),
                 ("n", NCHUNKS[1]), ("x", XCHUNKS[1]),   # d4, d3
                 ("n", NCHUNKS[2]),                      # d2, d1
                 ("x", XCHUNKS[2]), ("x", XCHUNKS[3])]   # d1, d0
        for kind, rng in loads:
            if kind == "w":
                nc.sync.dma_start(out=wp_t[:], in_=wpack[:])
            elif kind == "b":
                nc.sync.dma_start(out=bs_t[:], in_=bsT[:])
            elif kind == "x":
                d0, d1 = rng
                nc.sync.dma_start(
                    out=xt_t[d0][:],
                    in_=xrowsT[:, d0 * GROUP_PAD:d1 * GROUP_PAD])
            else:
                d0, d1 = rng
                nc.sync.dma_start(out=nc_t[d0][:],
                                  in_=ncatT[:, RCOL[d0]:RCOL[d1]])

        def xt_ap(d):
            """Group-d self rows [112, GROUP_PAD] inside its load chunk."""
            d0 = next(c0 for (c0, c1) in XCHUNKS if c0 <= d < c1)
            off = (d - d0) * GROUP_PAD
            return xt_t[d0][:, off:off + GROUP_PAD]

        def ncat_ap(d, s, cols):
            """Neighbour region (d, s) columns `cols` inside its chunk."""
            d0 = next(c0 for (c0, c1) in NCHUNKS if c0 <= d < c1)
            off = RCOL[d] - RCOL[d0] + s * GROUP_PAD
            return nc_t[d0][:, off + cols.start:off + cols.stop]

        # stores: 2-group for the early (heavy) groups, single-group for the
        # final two so the drain tail after the last relu is short
        STORES = {4: (4, 6), 2: (2, 4), 1: (1, 2), 0: (0, 1)}
        out_t = {}
        for od, (d0, d1) in STORES.items():
            out_t[od] = work_pool.tile([CONV, (d1 - d0) * GROUP_PAD], _bf16,
                                       tag=f"out{od}", name=f"out{od}")

        def out_ap(d):
            od = next(o for o, (c0, c1) in STORES.items() if c0 <= d < c1)
            off = (d - STORES[od][0]) * GROUP_PAD
            return out_t[od], od, off

        for i, d in enumerate(DORDER):
            xt = xt_ap(d)
            for s in range(min(d, S_DVE)):
                nc.vector.tensor_add(
                    xt[0:F_ATOM, :], xt[0:F_ATOM, :],
                    ncat_ap(d, s, slice(0, GROUP_PAD)))
            out_g, od, ocol = out_ap(d)
            n_pe = max(0, d - S_DVE)
            wmain = wp_t[:, d * CONV:(d + 1) * CONV]
            watom = wp_t[0:F_ATOM, d * CONV:(d + 1) * CONV]
            for q in range(NQ):
                cols = slice(q * QW, (q + 1) * QW)
                ps = ps_pool.tile([CONV, QW], _f32, tag="ps")
                nc.tensor.matmul(out=ps[:], lhsT=wmain, rhs=xt[:, cols],
                                 start=True, stop=(n_pe == 0))
                for j, s in enumerate(range(S_DVE, d)):
                    nc.tensor.matmul(out=ps[:], lhsT=watom,
                                     rhs=ncat_ap(d, s, cols),
                                     start=False, stop=(j == n_pe - 1))
                nc.scalar.activation(out_g[:, ocol + cols.start:
                                           ocol + cols.stop], ps[:],
                                     mybir.ActivationFunctionType.Relu,
                                     bias=bs_t[:, d:d + 1])
            if d == od:
                c0, c1 = STORES[od]
                nc.sync.dma_start(
                    out=osortT[:, c0 * GROUP_PAD:c1 * GROUP_PAD],
                    in_=out_t[od][:])

    nc.compile()
    return nc


def _get_program():
    if "nc" not in _cached:
        _cached["nc"] = build_program()
    return _cached["nc"]


def prep_core_inputs(atoms_s, bonds_s, edges_s, wpack_np, bsT_np):
    """Host-side layout/index prep for one core's shard (numpy only)."""
    deg = (edges_s != -1).sum(axis=-1).reshape(-1)            # [T] natural
    slot_tok = np.full(NSORT, -1, np.int64)   # sorted slot -> natural token
    for d in range(D):
        toks = np.nonzero(deg == d)[0]
        n = len(toks)
        assert n <= GROUP_PAD, f"degree-{d} group has {n} > {GROUP_PAD}"
        slot_tok[d * GROUP_PAD:d * GROUP_PAD + n] = toks

    flat = np.concatenate(
        [atoms_s.reshape(T, F_ATOM), bonds_s.reshape(T, D * F_BOND)], axis=1
    ).astype(ml_dtypes.bfloat16)                              # [T, 112]
    safe = np.maximum(slot_tok, 0)
    xrows = np.where((slot_tok >= 0)[:, None], flat[safe],
                     ml_dtypes.bfloat16(0))                   # [NSORT, 112]
    xrowsT = np.ascontiguousarray(xrows.T)                    # [112, NSORT]

    eflat = edges_s.reshape(T, D)
    bcol = (np.arange(T) // A) * A                            # molecule base
    atoms_flat = flat[:, :F_ATOM]
    regions = []
    for d in range(D):
        slots = slot_tok[d * GROUP_PAD:(d + 1) * GROUP_PAD]
        sv = slots >= 0
        st = np.maximum(slots, 0)
        for s in range(d):
            e = np.where(sv, eflat[st, s], -1)
            nat = np.maximum(bcol[st] + e, 0)
            regions.append(np.where((e >= 0)[:, None], atoms_flat[nat],
                                    ml_dtypes.bfloat16(0)))   # [2560, 64]
    ncat = np.concatenate(regions, axis=0)                    # [38400, 64]
    ncatT = np.ascontiguousarray(ncat.T)                      # [64, 38400]

    return {
        "xrowsT": xrowsT,
        "ncatT": ncatT,
        "wpack": wpack_np,
        "bsT": bsT_np,
    }, slot_tok


def kernel(atoms, bonds, edges, Ws, bs, trace=False):
    atoms = np.asarray(atoms)
    bonds = np.asarray(bonds)
    edges = np.asarray(edges)
    Ws = np.asarray(Ws)
    bs = np.asarray(bs)

    # wpack[:, d*128:(d+1)*128] = [Wa_d (64) | tile(Wb_d, 6) (48)]; the 6x
    # tiling makes the matmul itself perform the bond sum.
    wfull = np.zeros((D, ROW, CONV), np.float32)
    wfull[:, :F_ATOM] = Ws[:, :F_ATOM]
    wfull[:, F_ATOM:] = np.tile(Ws[:, F_ATOM:], (1, D, 1))
    wpack_np = np.ascontiguousarray(
        wfull.transpose(1, 0, 2).reshape(ROW, D * CONV)
    ).astype(ml_dtypes.bfloat16)
    bsT_np = np.ascontiguousarray(bs.T.astype(np.float32))    # [128, 6]

    in_maps, slot_toks = [], []
    for c in range(NCORES):
        sl = slice(c * BS, (c + 1) * BS)
        m, st = prep_core_inputs(atoms[sl], bonds[sl], edges[sl],
                                 wpack_np, bsT_np)
        in_maps.append(m)
        slot_toks.append(st)

    nc = _get_program()
    res = run_bass_kernel_spmd(nc, in_maps, core_ids=list(range(NCORES)),
                               trace=trace)
    kernel.last_results = res

    out = np.zeros((B, A, CONV), np.float32)
    for c in range(NCORES):
        osortT = res.results[c]["osortT"].view(ml_dtypes.bfloat16)
        osort = osortT.reshape(CONV, NSORT).T                 # [NSORT, 128]
        st = slot_toks[c]
        real = st >= 0
        shard = out[c * BS:(c + 1) * BS].reshape(T, CONV)
        shard[st[real]] = osort[real].astype(np.float32)
    return out
